# revision 22
# baseline (speedup 1.0000x reference)
"""Trainium2 Bass kernel for per-atom MLP grouped GEMM (moe_routing).

Problem: e[s,a] = MLP_a(g[s,a,:]) for S=2000 structs, A=1000 atoms,
each atom owning a tiny 5->32->32->1 tanh MLP.

Strategy:
  - Shard atoms across 8 cores (125 atoms/core, padded to 128).
  - Per core: 8 groups of 16 atoms; each group's 16 atoms are mapped to a
    4x4 grid of 32x32 PE sub-tiles (tile_position packing) so 16 tiny
    matmuls run concurrently on the 128x128 systolic array.
  - Layer biases: b1 folded in as an extra contraction row (ones row in g),
    b2 via a K=1 ones-matmul accumulated into the same PSUM group,
    b3 added on the host.
  - tanh on the scalar engine (ACT), reading [128, 2048] PSUM spans.
  - bf16 operands for the matmuls (PE streams 1 col/cycle), fp32 PSUM accum.
"""

import os
import sys

sys.path.insert(0, "/opt/trn_rl_repo")

import numpy as np
import ml_dtypes
from contextlib import ExitStack

import concourse.bass as bass
import concourse.tile as tile
from concourse import bacc, mybir
from concourse.bass_utils import run_bass_kernel_spmd

# ---- problem constants (hardcoded; kernel.py must be self-contained) ----
S, A, I, H = 2000, 1000, 5, 32
N_CORES = 8
A_PC = 128          # atoms per core, padded from 125
A_REAL = 125        # real atoms per core
G = 8               # atom groups per core (16 atoms each)
NS = 512            # struct tile (free dim per matmul)
ST = 4              # struct tiles (S padded to 2048)
S_PAD = NS * ST
K1 = 6              # L1 contraction: 5 inputs + 1 ones row (bias)
WCOLS = 400         # per-group weight columns: 128 W1 | 128 W2 | 128 b2 | 16 W3

BF16 = mybir.dt.bfloat16
FP32 = mybir.dt.float32

_cached = {}


def _build_program(repeat=1, detect_races=True, sim_safe=False):
    """Build the single-core SPMD bass program (same for all 8 cores).
    repeat>1 re-runs the whole computation (for marginal-time benchmarking).
    detect_races=False relaxes CoreSim's stale-read checker (the e-copy
    intentionally reads garbage rows of a reused PSUM slot)."""
    nc = bacc.Bacc(
        "TRN2",
        target_bir_lowering=False,
        debug=False,
        detect_race_conditions=detect_races,
    )
    gp = nc.dram_tensor("gp", [G, 4, K1, ST, 4, NS], BF16, kind="ExternalInput").ap()
    wp = nc.dram_tensor("wp", [G, 128, WCOLS], BF16, kind="ExternalInput").ap()
    eo = nc.dram_tensor("eo", [G, ST, 4, 2048], FP32, kind="ExternalOutput").ap()

    with tile.TileContext(nc) as tc:
        with ExitStack() as ctx:
            wpool = ctx.enter_context(tc.tile_pool(name="w", bufs=1))
            opool = ctx.enter_context(tc.tile_pool(name="ones", bufs=1))
            gpool = ctx.enter_context(tc.tile_pool(name="g", bufs=3))
            hpool = ctx.enter_context(tc.tile_pool(name="h", bufs=6))
            epool = ctx.enter_context(tc.tile_pool(name="e", bufs=3))
            pspool = ctx.enter_context(
                tc.tile_pool(name="ps", bufs=2, space="PSUM")
            )

            # persistent weights for all groups: [128, G*400]
            wt = wpool.tile([128, G * WCOLS], BF16)
            for g in range(G):
                nc.sync.dma_start(wt[:, g * WCOLS : (g + 1) * WCOLS], wp[g])

            ones = opool.tile([128, NS], BF16)
            nc.gpsimd.memset(ones[:], 1.0)

            FW = ST * 2112  # per-stile free blocks padded to 2112
            for g in [gg for _ in range(repeat) for gg in range(G)]:
                wg = g * WCOLS
                # one g tile per group; per-i DMAs keep partition APs 2D
                # (multi-partition-dim write APs break Tile dep tracking)
                gt = gpool.tile([128, FW], BF16)
                gtr = gt.rearrange("p (s f) -> p s f", f=2112)
                for i in range(4):
                    nc.sync.dma_start(
                        gtr[32 * i : 32 * i + K1, :, 0:2048], gp[g, i]
                    )
                for st in range(ST):

                    # ---- L1: 16 tiles (i,j): K=6, M=32, N=512 ----
                    ps1 = pspool.tile([128, 2048], FP32, tag="mm")
                    for j in range(4):
                        for i in range(4):
                            nc.tensor.matmul(
                                ps1[32 * j : 32 * j + 32, i * NS : (i + 1) * NS],
                                lhsT=wt[32 * i : 32 * i + K1, wg + j * 32 : wg + j * 32 + 32],
                                rhs=gt[32 * i : 32 * i + K1, st * 2112 + j * NS : st * 2112 + (j + 1) * NS],
                                start=True,
                                stop=True,
                                tile_position=(32 * i, 32 * j),
                            )
                    h1 = hpool.tile([128, 2048], BF16, tag="h")
                    nc.scalar.activation(
                        h1[:], ps1[:], mybir.ActivationFunctionType.Tanh
                    )

                    # ---- L2: 16 tiles (j,i): bias K=1 then W2 K=32 ----
                    ps2 = pspool.tile([128, 2048], FP32, tag="mm")
                    # bias/data pairs adjacent (PSUM accumulation group must
                    # close before another opens in the same bank region);
                    # j (= row group) varies fastest across pairs for LDW
                    # pull-ahead
                    for i in range(4):
                        for j in range(4):
                            nc.tensor.matmul(
                                ps2[32 * i : 32 * i + 32, j * NS : (j + 1) * NS],
                                lhsT=wt[32 * j : 32 * j + 1, wg + 256 + i * 32 : wg + 256 + i * 32 + 32],
                                rhs=ones[32 * j : 32 * j + 1, :],
                                start=True,
                                stop=False,
                                tile_position=(32 * j, 32 * i),
                            )
                            nc.tensor.matmul(
                                ps2[32 * i : 32 * i + 32, j * NS : (j + 1) * NS],
                                lhsT=wt[32 * j : 32 * j + 32, wg + 128 + i * 32 : wg + 128 + i * 32 + 32],
                                rhs=h1[32 * j : 32 * j + 32, i * NS : (i + 1) * NS],
                                start=False,
                                stop=True,
                                tile_position=(32 * j, 32 * i),
                            )
                    h2 = hpool.tile([128, 2048], BF16, tag="h")
                    nc.scalar.activation(
                        h2[:], ps2[:], mybir.ActivationFunctionType.Tanh
                    )

                    # ---- L3: 16 tiles (i,j): K=32, M=1 ----
                    # L3: e(i,j) -> [partition 32j, free i*NS] (out base
                    # must equal the col-group base; walrus enforces)
                    pse = pspool.tile([128, 2048], FP32, tag="mm")
                    for j in range(4):
                        for i in range(4):
                            nc.tensor.matmul(
                                pse[32 * j : 32 * j + 1, i * NS : (i + 1) * NS],
                                lhsT=wt[32 * i : 32 * i + 32, wg + 384 + 4 * i + j : wg + 384 + 4 * i + j + 1],
                                rhs=h2[32 * i : 32 * i + 32, j * NS : (j + 1) * NS],
                                start=True,
                                stop=True,
                                tile_position=(32 * i, 32 * j),
                            )
                    # DVE-copy e rows to SBUF (DMA cannot read PSUM).
                    # The full-width copy also reads stale rows of the
                    # reused slot (harmless; only rows {32j} are shipped) -
                    # sim_safe does 4 exact row copies to satisfy CoreSim's
                    # stale-read checker.
                    et = epool.tile([128, 2048], FP32, tag="e")
                    if sim_safe:
                        for j in range(4):
                            nc.vector.tensor_copy(
                                et[32 * j : 32 * j + 1, :],
                                pse[32 * j : 32 * j + 1, :],
                            )
                    else:
                        nc.vector.tensor_copy(et[:], pse[:])
                    et4 = et.rearrange("(a b) f -> a b f", b=32)
                    nc.gpsimd.dma_start(eo[g, st], et4[:, 0:1, :])

    nc.compile()
    return nc


def _pack_inputs(g, W1, b1, W2, b2, W3):
    """Pack full inputs into per-core DRAM layouts. Returns list of in_maps."""
    bf = ml_dtypes.bfloat16
    in_maps = []
    for c in range(N_CORES):
        a0 = c * A_REAL
        # atom index per (grp, i, j): a_local = 16*grp + 4*i + j
        gp = np.zeros((G, 4, K1, ST, 4, NS), dtype=bf)
        wp = np.zeros((G, 128, WCOLS), dtype=bf)
        # gather this core's real atoms
        for grp in range(G):
            for i in range(4):
                for j in range(4):
                    al = 16 * grp + 4 * i + j
                    if al >= A_REAL:
                        continue
                    a = a0 + al
                    # g: [S, A, I] -> gp[grp, i, k<5, st, j, s]
                    gs = np.zeros((S_PAD, I), dtype=np.float32)
                    gs[:S] = g[:, a, :]
                    gp[grp, i, :I, :, j, :] = (
                        gs.reshape(ST, NS, I).transpose(2, 0, 1).astype(bf)
                    )
                    gp[grp, i, I, :, j, :] = bf(1.0)  # ones row for b1
                    # W1 lhsT: rows 32i+k (k<6), cols j*32+h
                    wp[grp, 32 * i : 32 * i + I, j * 32 : j * 32 + 32] = W1[a].astype(bf)
                    wp[grp, 32 * i + I, j * 32 : j * 32 + 32] = b1[a].astype(bf)
                    # W2 lhsT: rows 32j+k, cols 128 + i*32+h
                    wp[grp, 32 * j : 32 * j + 32, 128 + i * 32 : 128 + i * 32 + 32] = W2[a].astype(bf)
                    # b2 lhsT: row 32j, cols 256 + i*32+h
                    wp[grp, 32 * j, 256 + i * 32 : 256 + i * 32 + 32] = b2[a].astype(bf)
                    # W3 lhsT: rows 32i+k, col 384 + 4i+j
                    wp[grp, 32 * i : 32 * i + 32, 384 + 4 * i + j] = W3[a, :, 0].astype(bf)
        in_maps.append({"gp": gp, "wp": wp})
    return in_maps


def _unpack_outputs(results, b3):
    """Assemble [S, A] output from per-core eo tensors; add b3 on host."""
    out = np.empty((S, A), dtype=np.float32)
    for c in range(N_CORES):
        e = results[c]["eo"].reshape(G, ST, 4, 4, NS)  # [grp, st, j, i, s]
        # -> [st*s, grp, i, j] -> [S_PAD, 128]
        e = e.transpose(1, 4, 0, 3, 2).reshape(S_PAD, G * 16)
        out[:, c * A_REAL : (c + 1) * A_REAL] = e[:S, :A_REAL]
    out += b3[None, :, 0]
    return out


def _make_runner(nc):
    """Build a reusable jitted SPMD callable (mirrors bass2jax.run_bass_via_pjrt
    but caches the jitted function so repeated calls don't re-trace)."""
    import jax
    from jax.sharding import Mesh, PartitionSpec
    from jax.experimental.shard_map import shard_map
    from concourse import bass2jax
    from concourse.bass2jax import (
        _bass_exec_p,
        install_neuronx_cc_hook,
        partition_id_tensor,
    )

    install_neuronx_cc_hook()

    partition_name = nc.partition_id_tensor.name if nc.partition_id_tensor else None
    in_names, out_names, out_avals = [], [], []
    for alloc in nc.m.functions[0].allocations:
        if not isinstance(alloc, mybir.MemoryLocationSet):
            continue
        name = alloc.memorylocations[0].name
        if alloc.kind == "ExternalInput":
            if name == partition_name:
                continue
            in_names.append(name)
        elif alloc.kind == "ExternalOutput":
            out_names.append(name)
            out_avals.append(
                jax.core.ShapedArray(
                    tuple(alloc.tensor_shape), mybir.dt.np(alloc.dtype)
                )
            )
    n_params = len(in_names)
    n_outs = len(out_avals)
    all_names = in_names + out_names
    if partition_name is not None:
        all_names = all_names + [partition_name]

    def _body(*args):
        operands = list(args)
        if partition_name is not None:
            operands.append(partition_id_tensor())
        outs = _bass_exec_p.bind(
            *operands,
            out_avals=tuple(out_avals),
            in_names=tuple(all_names),
            out_names=tuple(out_names),
            lowering_input_output_aliases=(),
            sim_require_finite=True,
            sim_require_nnan=True,
            nc=nc,
        )
        return tuple(outs)

    devices = jax.devices()[:N_CORES]
    mesh = Mesh(np.asarray(devices), ("core",))
    from jax.sharding import NamedSharding
    nspec = NamedSharding(mesh, PartitionSpec("core"))
    in_specs = (PartitionSpec("core"),) * (n_params + n_outs)
    out_specs = (PartitionSpec("core"),) * n_outs
    sharded = jax.jit(
        shard_map(_body, mesh=mesh, in_specs=in_specs, out_specs=out_specs,
                  check_rep=False),
        keep_unused=True,
    )

    def device_put_inputs(in_maps):
        arrs = [
            jax.device_put(
                np.concatenate([np.asarray(m[name]) for m in in_maps], axis=0),
                nspec,
            )
            for name in in_names
        ]
        # zero output-buffer operands, device-resident, reused (not donated)
        arrs += [
            jax.device_put(
                np.zeros((N_CORES * a.shape[0], *a.shape[1:]), a.dtype), nspec
            )
            for a in out_avals
        ]
        return arrs

    def run_device(concat_in):
        return sharded(*concat_in)

    def run(in_maps):
        out_arrs = sharded(*device_put_inputs(in_maps))
        return [
            {
                name: np.asarray(out_arrs[i]).reshape(
                    N_CORES, *out_avals[i].shape
                )[c]
                for i, name in enumerate(out_names)
            }
            for c in range(N_CORES)
        ], out_arrs

    run.device_put_inputs = device_put_inputs
    run.run_device = run_device
    return run


def get_runner():
    if "run" not in _cached:
        _cached["nc"] = _build_program()
        _cached["run"] = _make_runner(_cached["nc"])
    return _cached["run"]


def kernel(g, W1, b1, W2, b2, W3, b3):
    g = np.asarray(g, dtype=np.float32)
    W1 = np.asarray(W1, dtype=np.float32)
    b1 = np.asarray(b1, dtype=np.float32)
    W2 = np.asarray(W2, dtype=np.float32)
    b2 = np.asarray(b2, dtype=np.float32)
    W3 = np.asarray(W3, dtype=np.float32)
    b3 = np.asarray(b3, dtype=np.float32)

    run = get_runner()
    in_maps = _pack_inputs(g, W1, b1, W2, b2, W3)
    results, _ = run(in_maps)
    return _unpack_outputs(results, b3)


if __name__ == "__main__":
    # quick self-test against a small numpy model
    rng = np.random.default_rng(0)
    g = rng.standard_normal((S, A, I), dtype=np.float32)
    W1 = rng.standard_normal((A, I, H), dtype=np.float32) * 0.45
    b1 = rng.standard_normal((A, H), dtype=np.float32) * 0.01
    W2 = rng.standard_normal((A, H, H), dtype=np.float32) * 0.18
    b2 = rng.standard_normal((A, H), dtype=np.float32) * 0.01
    W3 = rng.standard_normal((A, H, 1), dtype=np.float32) * 0.18
    b3 = rng.standard_normal((A, 1), dtype=np.float32) * 0.01
    out = kernel(g, W1, b1, W2, b2, W3, b3)
    h1 = np.tanh(np.einsum("sai,aih->sah", g, W1) + b1[None])
    h2 = np.tanh(np.einsum("sah,aho->sao", h1, W2) + b2[None])
    ref = (np.einsum("sah,aho->sao", h2, W3) + b3[None])[..., 0]
    rel = np.abs(out - ref).max() / np.abs(ref).max()
    print("max rel err:", rel)


# revision 23
# speedup vs baseline: 2.4212x; 2.4212x over previous
"""Trainium2 Bass kernel for per-atom MLP grouped GEMM (moe_routing).

Problem: e[s,a] = MLP_a(g[s,a,:]) for S=2000 structs, A=1000 atoms,
each atom owning a tiny 5->32->32->1 tanh MLP.

Strategy:
  - Shard atoms across 8 cores (125 atoms/core, padded to 128).
  - Per core: 8 groups of 16 atoms; each group's 16 atoms are mapped to a
    4x4 grid of 32x32 PE sub-tiles (tile_position packing) so 16 tiny
    matmuls run concurrently on the 128x128 systolic array.
  - Layer biases: b1 folded in as an extra contraction row (ones row in g),
    b2 via a K=1 ones-matmul accumulated into the same PSUM group,
    b3 added on the host.
  - tanh on the scalar engine (ACT), reading [128, 2048] PSUM spans.
  - bf16 operands for the matmuls (PE streams 1 col/cycle), fp32 PSUM accum.
"""

import os
import sys

sys.path.insert(0, "/opt/trn_rl_repo")

import numpy as np
import ml_dtypes
from contextlib import ExitStack

import concourse.bass as bass
import concourse.tile as tile
from concourse import bacc, mybir
from concourse.bass_utils import run_bass_kernel_spmd

# ---- problem constants (hardcoded; kernel.py must be self-contained) ----
S, A, I, H = 2000, 1000, 5, 32
N_CORES = 8
A_PC = 128          # atoms per core, padded from 125
A_REAL = 125        # real atoms per core
G = 8               # atom groups per core (16 atoms each)
NS = 512            # struct tile (free dim per matmul)
ST = 4              # struct tiles (S padded to 2048)
S_PAD = NS * ST
K1 = 6              # L1 contraction: 5 inputs + 1 ones row (bias)
WCOLS = 400         # per-group weight columns: 128 W1 | 128 W2 | 128 b2 | 16 W3

BF16 = mybir.dt.bfloat16
FP32 = mybir.dt.float32

_cached = {}


def _build_program(repeat=1, detect_races=True, sim_safe=False):
    """Build the single-core SPMD bass program (same for all 8 cores).
    repeat>1 re-runs the whole computation (for marginal-time benchmarking).
    detect_races=False relaxes CoreSim's stale-read checker (the e-copy
    intentionally reads garbage rows of a reused PSUM slot)."""
    nc = bacc.Bacc(
        "TRN2",
        target_bir_lowering=False,
        debug=False,
        detect_race_conditions=detect_races,
    )
    gp = nc.dram_tensor("gp", [G, 4, K1, ST, 4, NS], BF16, kind="ExternalInput").ap()
    wp = nc.dram_tensor("wp", [G, 128, WCOLS], BF16, kind="ExternalInput").ap()
    eo = nc.dram_tensor("eo", [G, ST, 4, 2048], FP32, kind="ExternalOutput").ap()

    with tile.TileContext(nc) as tc:
        with ExitStack() as ctx:
            wpool = ctx.enter_context(tc.tile_pool(name="w", bufs=1))
            opool = ctx.enter_context(tc.tile_pool(name="ones", bufs=1))
            gpool = ctx.enter_context(tc.tile_pool(name="g", bufs=3))
            hpool = ctx.enter_context(tc.tile_pool(name="h", bufs=6))
            epool = ctx.enter_context(tc.tile_pool(name="e", bufs=3))
            pspool = ctx.enter_context(
                tc.tile_pool(name="ps", bufs=2, space="PSUM")
            )

            # persistent weights for all groups: [128, G*400]
            wt = wpool.tile([128, G * WCOLS], BF16)
            for g in range(G):
                nc.sync.dma_start(wt[:, g * WCOLS : (g + 1) * WCOLS], wp[g])

            ones = opool.tile([128, NS], BF16)
            nc.gpsimd.memset(ones[:], 1.0)

            FW = ST * 2112  # per-stile free blocks padded to 2112
            for g in [gg for _ in range(repeat) for gg in range(G)]:
                wg = g * WCOLS
                # one g tile per group; per-i DMAs keep partition APs 2D
                # (multi-partition-dim write APs break Tile dep tracking)
                gt = gpool.tile([128, FW], BF16)
                gtr = gt.rearrange("p (s f) -> p s f", f=2112)
                for i in range(4):
                    nc.sync.dma_start(
                        gtr[32 * i : 32 * i + K1, :, 0:2048], gp[g, i]
                    )
                for st in range(ST):

                    # ---- L1: 16 tiles (i,j): K=6, M=32, N=512 ----
                    ps1 = pspool.tile([128, 2048], FP32, tag="mm")
                    for j in range(4):
                        for i in range(4):
                            nc.tensor.matmul(
                                ps1[32 * j : 32 * j + 32, i * NS : (i + 1) * NS],
                                lhsT=wt[32 * i : 32 * i + K1, wg + j * 32 : wg + j * 32 + 32],
                                rhs=gt[32 * i : 32 * i + K1, st * 2112 + j * NS : st * 2112 + (j + 1) * NS],
                                start=True,
                                stop=True,
                                tile_position=(32 * i, 32 * j),
                            )
                    h1 = hpool.tile([128, 2048], BF16, tag="h")
                    nc.scalar.activation(
                        h1[:], ps1[:], mybir.ActivationFunctionType.Tanh
                    )

                    # ---- L2: 16 tiles (j,i): bias K=1 then W2 K=32 ----
                    ps2 = pspool.tile([128, 2048], FP32, tag="mm")
                    # bias/data pairs adjacent (PSUM accumulation group must
                    # close before another opens in the same bank region);
                    # j (= row group) varies fastest across pairs for LDW
                    # pull-ahead
                    for i in range(4):
                        for j in range(4):
                            nc.tensor.matmul(
                                ps2[32 * i : 32 * i + 32, j * NS : (j + 1) * NS],
                                lhsT=wt[32 * j : 32 * j + 1, wg + 256 + i * 32 : wg + 256 + i * 32 + 32],
                                rhs=ones[32 * j : 32 * j + 1, :],
                                start=True,
                                stop=False,
                                tile_position=(32 * j, 32 * i),
                            )
                            nc.tensor.matmul(
                                ps2[32 * i : 32 * i + 32, j * NS : (j + 1) * NS],
                                lhsT=wt[32 * j : 32 * j + 32, wg + 128 + i * 32 : wg + 128 + i * 32 + 32],
                                rhs=h1[32 * j : 32 * j + 32, i * NS : (i + 1) * NS],
                                start=False,
                                stop=True,
                                tile_position=(32 * j, 32 * i),
                            )
                    h2 = hpool.tile([128, 2048], BF16, tag="h")
                    nc.scalar.activation(
                        h2[:], ps2[:], mybir.ActivationFunctionType.Tanh
                    )

                    # ---- L3: 16 tiles (i,j): K=32, M=1 ----
                    # L3: e(i,j) -> [partition 32j, free i*NS] (out base
                    # must equal the col-group base; walrus enforces)
                    pse = pspool.tile([128, 2048], FP32, tag="mm")
                    for j in range(4):
                        for i in range(4):
                            nc.tensor.matmul(
                                pse[32 * j : 32 * j + 1, i * NS : (i + 1) * NS],
                                lhsT=wt[32 * i : 32 * i + 32, wg + 384 + 4 * i + j : wg + 384 + 4 * i + j + 1],
                                rhs=h2[32 * i : 32 * i + 32, j * NS : (j + 1) * NS],
                                start=True,
                                stop=True,
                                tile_position=(32 * i, 32 * j),
                            )
                    # DVE-copy e rows to SBUF (DMA cannot read PSUM).
                    # The full-width copy also reads stale rows of the
                    # reused slot (harmless; only rows {32j} are shipped) -
                    # sim_safe does 4 exact row copies to satisfy CoreSim's
                    # stale-read checker.
                    et = epool.tile([128, 2048], FP32, tag="e")
                    if sim_safe:
                        for j in range(4):
                            nc.vector.tensor_copy(
                                et[32 * j : 32 * j + 1, :],
                                pse[32 * j : 32 * j + 1, :],
                            )
                    else:
                        nc.vector.tensor_copy(et[:], pse[:])
                    et4 = et.rearrange("(a b) f -> a b f", b=32)
                    nc.sync.dma_start(eo[g, st], et4[:, 0:1, :])

    nc.compile()
    return nc


def _pack_inputs(g, W1, b1, W2, b2, W3):
    """Pack full inputs into per-core DRAM layouts. Returns list of in_maps."""
    bf = ml_dtypes.bfloat16
    in_maps = []
    for c in range(N_CORES):
        a0 = c * A_REAL
        # atom index per (grp, i, j): a_local = 16*grp + 4*i + j
        gp = np.zeros((G, 4, K1, ST, 4, NS), dtype=bf)
        wp = np.zeros((G, 128, WCOLS), dtype=bf)
        # gather this core's real atoms
        for grp in range(G):
            for i in range(4):
                for j in range(4):
                    al = 16 * grp + 4 * i + j
                    if al >= A_REAL:
                        continue
                    a = a0 + al
                    # g: [S, A, I] -> gp[grp, i, k<5, st, j, s]
                    gs = np.zeros((S_PAD, I), dtype=np.float32)
                    gs[:S] = g[:, a, :]
                    gp[grp, i, :I, :, j, :] = (
                        gs.reshape(ST, NS, I).transpose(2, 0, 1).astype(bf)
                    )
                    gp[grp, i, I, :, j, :] = bf(1.0)  # ones row for b1
                    # W1 lhsT: rows 32i+k (k<6), cols j*32+h
                    wp[grp, 32 * i : 32 * i + I, j * 32 : j * 32 + 32] = W1[a].astype(bf)
                    wp[grp, 32 * i + I, j * 32 : j * 32 + 32] = b1[a].astype(bf)
                    # W2 lhsT: rows 32j+k, cols 128 + i*32+h
                    wp[grp, 32 * j : 32 * j + 32, 128 + i * 32 : 128 + i * 32 + 32] = W2[a].astype(bf)
                    # b2 lhsT: row 32j, cols 256 + i*32+h
                    wp[grp, 32 * j, 256 + i * 32 : 256 + i * 32 + 32] = b2[a].astype(bf)
                    # W3 lhsT: rows 32i+k, col 384 + 4i+j
                    wp[grp, 32 * i : 32 * i + 32, 384 + 4 * i + j] = W3[a, :, 0].astype(bf)
        in_maps.append({"gp": gp, "wp": wp})
    return in_maps


def _unpack_outputs(results, b3):
    """Assemble [S, A] output from per-core eo tensors; add b3 on host."""
    out = np.empty((S, A), dtype=np.float32)
    for c in range(N_CORES):
        e = results[c]["eo"].reshape(G, ST, 4, 4, NS)  # [grp, st, j, i, s]
        # -> [st*s, grp, i, j] -> [S_PAD, 128]
        e = e.transpose(1, 4, 0, 3, 2).reshape(S_PAD, G * 16)
        out[:, c * A_REAL : (c + 1) * A_REAL] = e[:S, :A_REAL]
    out += b3[None, :, 0]
    return out


def _make_runner(nc):
    """Build a reusable jitted SPMD callable (mirrors bass2jax.run_bass_via_pjrt
    but caches the jitted function so repeated calls don't re-trace)."""
    import jax
    from jax.sharding import Mesh, PartitionSpec
    from jax.experimental.shard_map import shard_map
    from concourse import bass2jax
    from concourse.bass2jax import (
        _bass_exec_p,
        install_neuronx_cc_hook,
        partition_id_tensor,
    )

    install_neuronx_cc_hook()

    partition_name = nc.partition_id_tensor.name if nc.partition_id_tensor else None
    in_names, out_names, out_avals = [], [], []
    for alloc in nc.m.functions[0].allocations:
        if not isinstance(alloc, mybir.MemoryLocationSet):
            continue
        name = alloc.memorylocations[0].name
        if alloc.kind == "ExternalInput":
            if name == partition_name:
                continue
            in_names.append(name)
        elif alloc.kind == "ExternalOutput":
            out_names.append(name)
            out_avals.append(
                jax.core.ShapedArray(
                    tuple(alloc.tensor_shape), mybir.dt.np(alloc.dtype)
                )
            )
    n_params = len(in_names)
    n_outs = len(out_avals)
    all_names = in_names + out_names
    if partition_name is not None:
        all_names = all_names + [partition_name]

    def _body(*args):
        operands = list(args)
        if partition_name is not None:
            operands.append(partition_id_tensor())
        outs = _bass_exec_p.bind(
            *operands,
            out_avals=tuple(out_avals),
            in_names=tuple(all_names),
            out_names=tuple(out_names),
            lowering_input_output_aliases=(),
            sim_require_finite=True,
            sim_require_nnan=True,
            nc=nc,
        )
        return tuple(outs)

    devices = jax.devices()[:N_CORES]
    mesh = Mesh(np.asarray(devices), ("core",))
    from jax.sharding import NamedSharding
    nspec = NamedSharding(mesh, PartitionSpec("core"))
    in_specs = (PartitionSpec("core"),) * (n_params + n_outs)
    out_specs = (PartitionSpec("core"),) * n_outs
    sharded = jax.jit(
        shard_map(_body, mesh=mesh, in_specs=in_specs, out_specs=out_specs,
                  check_rep=False),
        keep_unused=True,
    )

    def device_put_inputs(in_maps):
        arrs = [
            jax.device_put(
                np.concatenate([np.asarray(m[name]) for m in in_maps], axis=0),
                nspec,
            )
            for name in in_names
        ]
        # zero output-buffer operands, device-resident, reused (not donated)
        arrs += [
            jax.device_put(
                np.zeros((N_CORES * a.shape[0], *a.shape[1:]), a.dtype), nspec
            )
            for a in out_avals
        ]
        return arrs

    def run_device(concat_in):
        return sharded(*concat_in)

    def run(in_maps):
        out_arrs = sharded(*device_put_inputs(in_maps))
        return [
            {
                name: np.asarray(out_arrs[i]).reshape(
                    N_CORES, *out_avals[i].shape
                )[c]
                for i, name in enumerate(out_names)
            }
            for c in range(N_CORES)
        ], out_arrs

    run.device_put_inputs = device_put_inputs
    run.run_device = run_device
    return run


def get_runner():
    if "run" not in _cached:
        _cached["nc"] = _build_program()
        _cached["run"] = _make_runner(_cached["nc"])
    return _cached["run"]


def kernel(g, W1, b1, W2, b2, W3, b3):
    g = np.asarray(g, dtype=np.float32)
    W1 = np.asarray(W1, dtype=np.float32)
    b1 = np.asarray(b1, dtype=np.float32)
    W2 = np.asarray(W2, dtype=np.float32)
    b2 = np.asarray(b2, dtype=np.float32)
    W3 = np.asarray(W3, dtype=np.float32)
    b3 = np.asarray(b3, dtype=np.float32)

    run = get_runner()
    in_maps = _pack_inputs(g, W1, b1, W2, b2, W3)
    results, _ = run(in_maps)
    return _unpack_outputs(results, b3)


if __name__ == "__main__":
    # quick self-test against a small numpy model
    rng = np.random.default_rng(0)
    g = rng.standard_normal((S, A, I), dtype=np.float32)
    W1 = rng.standard_normal((A, I, H), dtype=np.float32) * 0.45
    b1 = rng.standard_normal((A, H), dtype=np.float32) * 0.01
    W2 = rng.standard_normal((A, H, H), dtype=np.float32) * 0.18
    b2 = rng.standard_normal((A, H), dtype=np.float32) * 0.01
    W3 = rng.standard_normal((A, H, 1), dtype=np.float32) * 0.18
    b3 = rng.standard_normal((A, 1), dtype=np.float32) * 0.01
    out = kernel(g, W1, b1, W2, b2, W3, b3)
    h1 = np.tanh(np.einsum("sai,aih->sah", g, W1) + b1[None])
    h2 = np.tanh(np.einsum("sah,aho->sao", h1, W2) + b2[None])
    ref = (np.einsum("sah,aho->sao", h2, W3) + b3[None])[..., 0]
    rel = np.abs(out - ref).max() / np.abs(ref).max()
    print("max rel err:", rel)
